# revision 7
# baseline (speedup 1.0000x reference)
"""Trainium2 Bass kernel for nn_CrossAttentionModule (cross-attention fusion).

Computation (per batch b):
  rgb_att = Attn(Q=conv(F_rgb,Wq_rgb), K=conv(F_ind,Wk_ind), V=conv(F_ind,Wv_ind))
  ind_att = Attn(Q=conv(F_ind,Wq_ind), K=conv(F_rgb,Wk_rgb), V=conv(F_rgb,Wv_rgb))
  fused   = w*rgb_att + (1-w)*ind_att
  returns fused, (F_rgb, F_indices), stack([rgb_att, ind_att], 1)

Sharding: 8 cores = 4 batches x 2 attention directions. Each core computes its
(b, dir) attention output; the fused output is formed with a pair-wise
AllReduce of per-core pre-scaled attention outputs (the scale coefficient,
w or 1-w, is supplied per-core as input data so the SPMD program is uniform).

Per-core compute layout (C=2048 channels, HW=1024 pixels, P=128):
  Q'[c,i] = sum_k WqT[k,c] Xq[k,i] + bq[c]        (out: [c-part, i-free])
  K'[c,j] likewise; V^T[j,c] = sum_k Xkv[k,j] WvT[k,c]   (V bias folded later)
  S^T[j,i] = sum_c K'[c,j] Q'[c,i]                 (psum accumulate over c)
  Pt = exp(S^T / sqrt(C))                          (unnormalized, softmax over j)
  colsum[i] broadcast to all partitions via ones[128,128] @ Pt matmuls
  O[c,i] = (sum_j V^T[j,c] Pt[j,i]) * recip_colsum[i] + bv[c]
  (exact: sum_j softmax == 1 folds the V bias into a per-partition add)

Matmuls run as float32r (FP22 truncation) which is full-rate on the PE for
free-dim >= 256 while keeping ~1e-4 relative accuracy.
"""

import math
import os
import sys

for _p in ("/opt/trn_rl_repo",):
    if _p not in sys.path:
        sys.path.insert(0, _p)

import numpy as np

import concourse.bass as bass
import concourse.mybir as mybir
import concourse.tile as tile
from concourse import bacc
from concourse.bass_utils import run_bass_kernel_spmd

B, C, H, W = 4, 2048, 32, 32
HW = H * W
P = 128
KT = C // P      # 16 contraction tiles
MT = C // P      # 16 output-channel tiles
JT = HW // P     # 8 pixel tiles (attention keys)
NF = 512         # matmul moving free dim
IC = HW // NF    # 2 i-chunks
N_CORES = 8
PAIRS = [[0, 1], [2, 3], [4, 5], [6, 7]]
N_GROUPS = 4     # fused allreduce chunks (4 m-tiles each)

# 'fp32r' (fp32 storage, FP22 matmul reads) or 'bf16'
DT_MODE = os.environ.get("KERNEL_DT", "fp32r")

_PROGRAM_CACHE = {}


def _np_dt(d):
    return mybir.dt.np(d)


def _build_program(mode: str):
    f32 = mybir.dt.float32
    if mode == "bf16":
        dt_in = mybir.dt.bfloat16   # dtype of x/w inputs in DRAM
        dt_sp = mybir.dt.bfloat16   # dtype of q/k/vt spills and Pt
    elif mode == "fp32r":
        dt_in = mybir.dt.float32r
        dt_sp = mybir.dt.float32r
    elif mode == "fp32":
        dt_in = f32
        dt_sp = f32
    else:
        raise ValueError(mode)

    def mc(ap):
        return ap

    nc = bacc.Bacc(
        "TRN2", target_bir_lowering=False, debug=False, num_devices=N_CORES
    )

    xq_d = nc.dram_tensor("xq", [C, HW], dt_in, kind="ExternalInput").ap()
    xkv_d = nc.dram_tensor("xkv", [C, HW], dt_in, kind="ExternalInput").ap()
    wq_d = nc.dram_tensor("wq", [C, C], dt_in, kind="ExternalInput").ap()
    wk_d = nc.dram_tensor("wk", [C, C], dt_in, kind="ExternalInput").ap()
    wv_d = nc.dram_tensor("wv", [C, C], dt_in, kind="ExternalInput").ap()
    bq_d = nc.dram_tensor("bq", [C], f32, kind="ExternalInput").ap()
    bk_d = nc.dram_tensor("bk", [C], f32, kind="ExternalInput").ap()
    bv_d = nc.dram_tensor("bv", [C], f32, kind="ExternalInput").ap()
    coef_d = nc.dram_tensor("coef", [P, 1], f32, kind="ExternalInput").ap()
    ones_d = nc.dram_tensor("ones", [P, P], dt_sp, kind="ExternalInput").ap()

    att_d = nc.dram_tensor("att", [C, HW], f32, kind="ExternalOutput").ap()
    fused_d = nc.dram_tensor("fused", [C, HW], f32, kind="ExternalOutput").ap()

    scale = 1.0 / math.sqrt(float(C))
    Exp = mybir.ActivationFunctionType.Exp
    Ident = mybir.ActivationFunctionType.Identity
    Copy = mybir.ActivationFunctionType.Copy

    with tile.TileContext(nc) as tc:
        with tc.tile_pool(name="dram", bufs=1, space="DRAM") as dram_pool, \
             tc.tile_pool(name="consts", bufs=1) as consts, \
             tc.tile_pool(name="psum", bufs=4, space="PSUM") as psum_pool, \
             tc.tile_pool(name="stage", bufs=4) as stage_pool:

            # spills
            qd = dram_pool.tile([C, HW], dt_sp, name="qd")
            kd = dram_pool.tile([C, HW], dt_sp, name="kd")
            vtd = dram_pool.tile([HW, C], dt_sp, name="vtd")
            ar_in = [
                dram_pool.tile([4 * P, HW], f32, name=f"ar_in{g}")
                for g in range(N_GROUPS)
            ]
            ar_out = [
                dram_pool.tile([4 * P, HW], f32, name=f"ar_out{g}")
                for g in range(N_GROUPS)
            ]

            # constants
            bias_sb = consts.tile([P, 3 * MT], f32, name="bias_sb")
            nc.sync.dma_start(bias_sb[:, 0:MT], bq_d.rearrange("(m p) -> p m", p=P))
            nc.sync.dma_start(bias_sb[:, MT:2 * MT], bk_d.rearrange("(m p) -> p m", p=P))
            nc.sync.dma_start(bias_sb[:, 2 * MT:3 * MT], bv_d.rearrange("(m p) -> p m", p=P))
            coef_sb = consts.tile([P, 1], f32, name="coef_sb")
            nc.sync.dma_start(coef_sb[:], coef_d[:])
            ones_sb = consts.tile([P, P], dt_sp, name="ones_sb")
            nc.sync.dma_start(ones_sb[:], ones_d[:])

            # ---------------- Stage A: projections ----------------
            def projection(x_sb, w_dram, bias_col, out_dram, out_is_qk):
                """out_is_qk: out[c_out, i]; else v-style out[j, c_out]."""
                w_r = w_dram.rearrange("(ko p) o -> p ko o", p=P)
                with tc.tile_pool(name="wpool", bufs=2) as wpool:
                    for oq in range(4):  # quarters of the 2048 output channels
                        wt = wpool.tile([P, KT, NF], dt_in, name="wt", tag="wt")
                        nc.sync.dma_start(wt[:], w_r[:, :, oq * NF:(oq + 1) * NF])
                        if out_is_qk:
                            # out tiles: m = output-channel tile, N = i chunk
                            for mi in range(4):
                                m = oq * 4 + mi
                                for ic in range(IC):
                                    ps = psum_pool.tile([P, NF], f32, name="ps_a", tag="ps_a")
                                    for k in range(KT):
                                        nc.tensor.matmul(
                                            ps[:],
                                            mc(wt[:, k, mi * P:(mi + 1) * P]),
                                            mc(x_sb[:, k, ic * NF:(ic + 1) * NF]),
                                            start=(k == 0),
                                            stop=(k == KT - 1),
                                        )
                                    st = stage_pool.tile([P, NF], dt_sp, name="st_a", tag="st_a")
                                    nc.scalar.activation(
                                        st[:], ps[:], Ident,
                                        bias=bias_sb[:, bias_col + m:bias_col + m + 1],
                                    )
                                    nc.sync.dma_start(
                                        out_dram[m * P:(m + 1) * P, ic * NF:(ic + 1) * NF],
                                        st[:],
                                    )
                        else:
                            # v-style: out[j, c_out]; lhsT = x tiles, rhs = w
                            for jt in range(JT):
                                ps = psum_pool.tile([P, NF], f32, name="ps_a", tag="ps_a")
                                for k in range(KT):
                                    nc.tensor.matmul(
                                        ps[:],
                                        mc(x_sb[:, k, jt * P:(jt + 1) * P]),
                                        mc(wt[:, k, :]),
                                        start=(k == 0),
                                        stop=(k == KT - 1),
                                    )
                                st = stage_pool.tile([P, NF], dt_sp, name="st_a", tag="st_a")
                                nc.scalar.activation(st[:], ps[:], Copy)
                                nc.sync.dma_start(
                                    out_dram[jt * P:(jt + 1) * P, oq * NF:(oq + 1) * NF],
                                    st[:],
                                )

            with tc.tile_pool(name="xpool", bufs=1) as xpool:
                xkv_sb = xpool.tile([P, KT, HW], dt_in, name="xkv_sb")
                nc.sync.dma_start(xkv_sb[:], xkv_d.rearrange("(ko p) i -> p ko i", p=P))
                projection(xkv_sb, wk_d, MT, kd, True)        # K'
                projection(xkv_sb, wv_d, 0, vtd, False)       # V^T (no bias here)
            with tc.tile_pool(name="xpool2", bufs=1) as xpool2:
                xq_sb = xpool2.tile([P, KT, HW], dt_in, name="xq_sb")
                nc.sync.dma_start(xq_sb[:], xq_d.rearrange("(ko p) i -> p ko i", p=P))
                projection(xq_sb, wq_d, 0, qd, True)          # Q'

            # ---------------- Stage B: S^T, exp, colsum ----------------
            with tc.tile_pool(name="cspool", bufs=1, space="PSUM") as cspool, \
                 tc.tile_pool(name="ppool", bufs=1) as ppool, \
                 tc.tile_pool(name="rpool", bufs=1) as rpool:
                pt = ppool.tile([P, JT, HW], dt_sp, name="pt")
                cs_ps = cspool.tile([P, HW], f32, name="cs_ps")

                with tc.tile_pool(name="qkpool", bufs=1) as qkpool:
                    qt = qkpool.tile([P, MT, HW], dt_sp, name="qt")
                    nc.sync.dma_start(qt[:], qd[:].rearrange("(mo p) i -> p mo i", p=P))
                    kt_sb = qkpool.tile([P, MT, HW], dt_sp, name="kt_sb")
                    nc.sync.dma_start(kt_sb[:], kd[:].rearrange("(mo p) i -> p mo i", p=P))

                    for jt in range(JT):
                        for ic in range(IC):
                            ps = psum_pool.tile([P, NF], f32, name="ps_s", tag="ps_a")
                            for m in range(MT):
                                nc.tensor.matmul(
                                    ps[:],
                                    mc(kt_sb[:, m, jt * P:(jt + 1) * P]),
                                    mc(qt[:, m, ic * NF:(ic + 1) * NF]),
                                    start=(m == 0),
                                    stop=(m == MT - 1),
                                )
                            p_slice = pt[:, jt, ic * NF:(ic + 1) * NF]
                            nc.scalar.activation(p_slice, ps[:], Exp, scale=scale)
                            nc.tensor.matmul(
                                cs_ps[:, ic * NF:(ic + 1) * NF],
                                mc(ones_sb[:]),
                                mc(p_slice),
                                start=(jt == 0),
                                stop=(jt == JT - 1),
                            )

                recip = rpool.tile([P, HW], f32, name="recip")
                nc.vector.reciprocal(recip[:], cs_ps[:])

                # ---------------- Stage C: O = V^T.T @ Pt, fuse ----------------
                with tc.tile_pool(name="vpool", bufs=1) as vpool, \
                     tc.tile_pool(name="opool", bufs=4) as opool:
                    vt = vpool.tile([P, JT, C], dt_sp, name="vt")
                    nc.sync.dma_start(vt[:], vtd[:].rearrange("(jo p) c -> p jo c", p=P))

                    for g in range(N_GROUPS):
                        for mi in range(4):
                            m = g * 4 + mi
                            for ic in range(IC):
                                ps = psum_pool.tile([P, NF], f32, name="ps_o", tag="ps_a")
                                for jt in range(JT):
                                    nc.tensor.matmul(
                                        ps[:],
                                        mc(vt[:, jt, m * P:(m + 1) * P]),
                                        mc(pt[:, jt, ic * NF:(ic + 1) * NF]),
                                        start=(jt == 0),
                                        stop=(jt == JT - 1),
                                    )
                                t1 = opool.tile([P, NF], f32, name="t1", tag="t1")
                                nc.vector.tensor_tensor(
                                    t1[:], ps[:], recip[:, ic * NF:(ic + 1) * NF],
                                    mybir.AluOpType.mult,
                                )
                                att_sb = opool.tile([P, NF], f32, name="att_sb", tag="att_sb")
                                nc.scalar.activation(
                                    att_sb[:], t1[:], Ident,
                                    bias=bias_sb[:, 2 * MT + m:2 * MT + m + 1],
                                )
                                nc.sync.dma_start(
                                    att_d[m * P:(m + 1) * P, ic * NF:(ic + 1) * NF],
                                    att_sb[:],
                                )
                                sc = opool.tile([P, NF], f32, name="sc", tag="sc")
                                nc.scalar.mul(sc[:], att_sb[:], coef_sb[:, 0:1])
                                nc.sync.dma_start(
                                    ar_in[g][mi * P:(mi + 1) * P, ic * NF:(ic + 1) * NF],
                                    sc[:],
                                )
                        nc.gpsimd.collective_compute(
                            "AllReduce",
                            mybir.AluOpType.add,
                            replica_groups=PAIRS,
                            ins=[ar_in[g][:].opt()],
                            outs=[ar_out[g][:].opt()],
                        )
                        nc.sync.dma_start(
                            fused_d[g * 4 * P:(g + 1) * 4 * P, :], ar_out[g][:]
                        )

    nc.compile()
    return nc


def _get_program(mode=None):
    mode = mode or DT_MODE
    if mode not in _PROGRAM_CACHE:
        _PROGRAM_CACHE[mode] = _build_program(mode)
    return _PROGRAM_CACHE[mode]


def _shard_inputs(inputs, mode=None):
    mode = mode or DT_MODE
    if mode == "bf16":
        import ml_dtypes
        np_in = ml_dtypes.bfloat16
    else:
        np_in = np.float32

    F_rgb = np.asarray(inputs["F_rgb"], dtype=np.float32).reshape(B, C, HW)
    F_ind = np.asarray(inputs["F_indices"], dtype=np.float32).reshape(B, C, HW)
    w = float(np.asarray(inputs["fusion_weight"], dtype=np.float32))

    def wt(name):
        a = np.asarray(inputs[name], dtype=np.float32)
        return np.ascontiguousarray(a.T).astype(np_in)

    wq_rgb, wq_ind = wt("w_q_rgb"), wt("w_q_ind")
    wk_rgb, wk_ind = wt("w_k_rgb"), wt("w_k_ind")
    wv_rgb, wv_ind = wt("w_v_rgb"), wt("w_v_ind")

    def bias(name):
        return np.ascontiguousarray(np.asarray(inputs[name], dtype=np.float32))

    ones = np.ones((P, P), dtype=np_in)
    in_maps = []
    for b in range(B):
        xr = np.ascontiguousarray(F_rgb[b]).astype(np_in)
        xi = np.ascontiguousarray(F_ind[b]).astype(np_in)
        # dir 0: rgb_att -- Q from rgb, K/V from indices
        in_maps.append({
            "xq": xr, "xkv": xi,
            "wq": wq_rgb, "wk": wk_ind, "wv": wv_ind,
            "bq": bias("b_q_rgb"), "bk": bias("b_k_ind"), "bv": bias("b_v_ind"),
            "coef": np.full((P, 1), w, dtype=np.float32),
            "ones": ones,
        })
        # dir 1: ind_att -- Q from indices, K/V from rgb
        in_maps.append({
            "xq": xi, "xkv": xr,
            "wq": wq_ind, "wk": wk_rgb, "wv": wv_rgb,
            "bq": bias("b_q_ind"), "bk": bias("b_k_rgb"), "bv": bias("b_v_rgb"),
            "coef": np.full((P, 1), 1.0 - w, dtype=np.float32),
            "ones": ones,
        })
    return in_maps


def _assemble(inputs, results):
    fused = np.empty((B, C, H, W), dtype=np.float32)
    attention_maps = np.empty((B, 2, C, H, W), dtype=np.float32)
    for b in range(B):
        attention_maps[b, 0] = results[2 * b]["att"].reshape(C, H, W)
        attention_maps[b, 1] = results[2 * b + 1]["att"].reshape(C, H, W)
        fused[b] = results[2 * b]["fused"].reshape(C, H, W)
    F_rgb = np.asarray(inputs["F_rgb"], dtype=np.float32)
    F_ind = np.asarray(inputs["F_indices"], dtype=np.float32)
    return fused, (F_rgb, F_ind), attention_maps


def run(inputs, mode=None, trace=False, tmpdir=None):
    nc = _get_program(mode)
    in_maps = _shard_inputs(inputs, mode)
    res = run_bass_kernel_spmd(
        nc, in_maps, core_ids=list(range(N_CORES)), trace=trace, tmpdir=tmpdir
    )
    return _assemble(inputs, res.results), res


def kernel(**inputs):
    out, _ = run(inputs)
    return out


# revision 9
# speedup vs baseline: 1.1334x; 1.1334x over previous
"""Trainium2 Bass kernel for nn_CrossAttentionModule (cross-attention fusion).

Computation (per batch b):
  rgb_att = Attn(Q=conv(F_rgb,Wq_rgb), K=conv(F_ind,Wk_ind), V=conv(F_ind,Wv_ind))
  ind_att = Attn(Q=conv(F_ind,Wq_ind), K=conv(F_rgb,Wk_rgb), V=conv(F_rgb,Wv_rgb))
  fused   = w*rgb_att + (1-w)*ind_att
  returns fused, (F_rgb, F_indices), stack([rgb_att, ind_att], 1)

Sharding: 8 cores = 4 batches x 2 attention directions. Each core computes its
(b, dir) attention output. For the fused output each pair of cores exchanges
the half of its attention output that the peer fuses (AllGather of the "send"
half), then both compute their fused half as a*gath[0] + b*gath[1] + c*keep
with host-supplied per-core coefficients (a,b,c), so the SPMD program is
uniform across cores. The V projection's output channels are host-permuted
([keep; send] order) per core parity; the host un-permutes the outputs.

Per-core compute (C=2048 channels, HW=1024 pixels, P=128):
  Q'[c,i] = sum_k WqT[k,c] Xq[k,i] + bq[c]        (out: [c-part, i-free])
  K'[c,j] likewise; V^T[j,c] = sum_k Xkv[k,j] WvT[k,c]   (V bias folded later)
  S^T[j,i] = sum_c K'[c,j] Q'[c,i]                 (psum accumulate over c)
  Pt = exp(S^T / sqrt(C))                          (unnormalized, softmax over j)
  colsum[i] broadcast to all partitions via ones[128,128] @ Pt matmuls
  O[c,i] = (sum_j V^T[j,c] Pt[j,i]) * recip_colsum[i] + bv[c]
  (exact: sum_j softmax == 1 folds the V bias into a per-partition add)

Matmuls run as float32r (FP22 truncated reads) which is full-rate on the PE
for free-dim >= 256 while keeping ~3e-4 relative accuracy.
"""

import math
import os
import sys

for _p in ("/opt/trn_rl_repo",):
    if _p not in sys.path:
        sys.path.insert(0, _p)

import numpy as np

import concourse.bass as bass
import concourse.mybir as mybir
import concourse.tile as tile
from concourse import bacc
from concourse.bass_utils import run_bass_kernel_spmd

B, C, H, W = 4, 2048, 32, 32
HW = H * W
P = 128
KT = C // P      # 16 contraction tiles
MT = C // P      # 16 output-channel tiles
JT = HW // P     # 8 pixel tiles (attention keys)
NF = 512         # matmul moving free dim
IC = HW // NF    # 2 i-chunks
N_CORES = 8
PAIRS = [[0, 1], [2, 3], [4, 5], [6, 7]]
HALF = C // 2

DT_MODE = os.environ.get("KERNEL_DT", "fp32r")  # 'fp32r' | 'bf16' | 'fp32'

_PROGRAM_CACHE = {}


def _build_program(mode: str):
    f32 = mybir.dt.float32
    if mode == "bf16":
        dt_c = mybir.dt.bfloat16
    elif mode == "fp32r":
        dt_c = mybir.dt.float32r
    elif mode == "fp32":
        dt_c = f32
    else:
        raise ValueError(mode)

    nc = bacc.Bacc(
        "TRN2", target_bir_lowering=False, debug=False, num_devices=N_CORES
    )

    xq_d = nc.dram_tensor("xq", [C, HW], dt_c, kind="ExternalInput").ap()
    xkv_d = nc.dram_tensor("xkv", [C, HW], dt_c, kind="ExternalInput").ap()
    wq_d = nc.dram_tensor("wq", [C, C], dt_c, kind="ExternalInput").ap()
    wk_d = nc.dram_tensor("wk", [C, C], dt_c, kind="ExternalInput").ap()
    wv_d = nc.dram_tensor("wv", [C, C], dt_c, kind="ExternalInput").ap()
    bq_d = nc.dram_tensor("bq", [C], f32, kind="ExternalInput").ap()
    bk_d = nc.dram_tensor("bk", [C], f32, kind="ExternalInput").ap()
    bv_d = nc.dram_tensor("bv", [C], f32, kind="ExternalInput").ap()
    coef_d = nc.dram_tensor("coef", [P, 3], f32, kind="ExternalInput").ap()
    ones_d = nc.dram_tensor("ones", [P, P], dt_c, kind="ExternalInput").ap()

    att_d = nc.dram_tensor("att", [C, HW], f32, kind="ExternalOutput").ap()
    fusedh_d = nc.dram_tensor("fusedh", [HALF, HW], f32, kind="ExternalOutput").ap()

    scale = 1.0 / math.sqrt(float(C))
    Exp = mybir.ActivationFunctionType.Exp
    Ident = mybir.ActivationFunctionType.Identity
    Copy = mybir.ActivationFunctionType.Copy

    with tile.TileContext(nc) as tc:
        with tc.tile_pool(name="dram", bufs=1, space="DRAM") as dram_pool, \
             tc.tile_pool(name="consts", bufs=1) as consts, \
             tc.tile_pool(name="psum", bufs=4, space="PSUM") as psum_pool, \
             tc.tile_pool(name="stage", bufs=4) as stage_pool:

            # DRAM spills and collective buffers
            qd = dram_pool.tile([C, HW], dt_c, name="qd")
            kd = dram_pool.tile([C, HW], dt_c, name="kd")
            vtd = dram_pool.tile([HW, C], dt_c, name="vtd")
            cc_in = [dram_pool.tile([4 * P, HW], f32, name=f"cc_in{g}")
                     for g in range(2)]
            cc_out = [dram_pool.tile([2, 4 * P, HW], f32, name=f"cc_out{g}")
                      for g in range(2)]

            # constants
            bias_sb = consts.tile([P, 3 * MT], f32, name="bias_sb")
            nc.sync.dma_start(bias_sb[:, 0:MT], bq_d.rearrange("(m p) -> p m", p=P))
            nc.sync.dma_start(bias_sb[:, MT:2 * MT], bk_d.rearrange("(m p) -> p m", p=P))
            nc.sync.dma_start(bias_sb[:, 2 * MT:3 * MT], bv_d.rearrange("(m p) -> p m", p=P))
            coef_sb = consts.tile([P, 3], f32, name="coef_sb")
            nc.sync.dma_start(coef_sb[:], coef_d[:])
            ones_sb = consts.tile([P, P], dt_c, name="ones_sb")
            nc.sync.dma_start(ones_sb[:], ones_d[:])

            # ---------------- Stage A: projections (spill to DRAM) ----------
            def projection(x_sb, w_dram, bias_col, out_dram, out_is_qk):
                """out_is_qk: out[c_out, i]; else v-style out[j, c_out]."""
                w_r = w_dram.rearrange("(ko p) o -> p ko o", p=P)
                with tc.tile_pool(name="wpool", bufs=2) as wpool:
                    for oq in range(4):  # quarters of the output channels
                        wt = wpool.tile([P, KT, NF], dt_c, name="wt", tag="wt")
                        nc.sync.dma_start(wt[:], w_r[:, :, oq * NF:(oq + 1) * NF])
                        if out_is_qk:
                            for mi in range(4):
                                m = oq * 4 + mi
                                for ic in range(IC):
                                    ps = psum_pool.tile([P, NF], f32, name="ps_a", tag="ps")
                                    for k in range(KT):
                                        nc.tensor.matmul(
                                            ps[:],
                                            wt[:, k, mi * P:(mi + 1) * P],
                                            x_sb[:, k, ic * NF:(ic + 1) * NF],
                                            start=(k == 0),
                                            stop=(k == KT - 1),
                                        )
                                    st = stage_pool.tile([P, NF], dt_c, name="st_a", tag="st")
                                    nc.scalar.activation(
                                        st[:], ps[:], Ident,
                                        bias=bias_sb[:, bias_col + m:bias_col + m + 1],
                                    )
                                    nc.sync.dma_start(
                                        out_dram[m * P:(m + 1) * P, ic * NF:(ic + 1) * NF],
                                        st[:],
                                    )
                        else:
                            for jt in range(JT):
                                ps = psum_pool.tile([P, NF], f32, name="ps_a", tag="ps")
                                for k in range(KT):
                                    nc.tensor.matmul(
                                        ps[:],
                                        x_sb[:, k, jt * P:(jt + 1) * P],
                                        wt[:, k, :],
                                        start=(k == 0),
                                        stop=(k == KT - 1),
                                    )
                                st = stage_pool.tile([P, NF], dt_c, name="st_a", tag="st")
                                nc.scalar.activation(st[:], ps[:], Copy)
                                nc.sync.dma_start(
                                    out_dram[jt * P:(jt + 1) * P, oq * NF:(oq + 1) * NF],
                                    st[:],
                                )

            def load_x(pool, src, name):
                x_sb = pool.tile([P, KT, HW], dt_c, name=name)
                src_r = src.rearrange("(ko p) i -> p ko i", p=P)
                for k in range(KT):
                    nc.sync.dma_start(x_sb[:, k, :], src_r[:, k, :])
                return x_sb

            with tc.tile_pool(name="xpool", bufs=1) as xpool:
                xkv_sb = load_x(xpool, xkv_d, "xkv_sb")
                projection(xkv_sb, wk_d, MT, kd, True)        # K'
                projection(xkv_sb, wv_d, 0, vtd, False)       # V^T (no bias)
            with tc.tile_pool(name="xpool2", bufs=1) as xpool2:
                xq_sb = load_x(xpool2, xq_d, "xq_sb")
                projection(xq_sb, wq_d, 0, qd, True)          # Q'

            # ---------------- Stage B: S^T, exp, colsum ---------------------
            with tc.tile_pool(name="cspool", bufs=1, space="PSUM") as cspool, \
                 tc.tile_pool(name="ppool", bufs=1) as ppool, \
                 tc.tile_pool(name="rpool", bufs=1) as rpool:
                pt = ppool.tile([P, JT, HW], dt_c, name="pt")
                cs_ps = cspool.tile([P, HW], f32, name="cs_ps")

                with tc.tile_pool(name="qkpool", bufs=1) as qkpool:
                    qt = qkpool.tile([P, MT, HW], dt_c, name="qt")
                    kt_sb = qkpool.tile([P, MT, HW], dt_c, name="kt_sb")
                    qd_r = qd[:].rearrange("(mo p) i -> p mo i", p=P)
                    kd_r = kd[:].rearrange("(mo p) i -> p mo i", p=P)
                    # per-m reload so S matmul chains can start while streaming
                    for m in range(MT):
                        nc.sync.dma_start(qt[:, m, :], qd_r[:, m, :])
                        nc.sync.dma_start(kt_sb[:, m, :], kd_r[:, m, :])

                    for ic in range(IC):
                        for pr in range(JT // 2):  # jt pairs
                            jts = (2 * pr, 2 * pr + 1)
                            pss = [
                                psum_pool.tile([P, NF], f32, name="ps_s", tag="ps")
                                for _ in jts
                            ]
                            for m in range(MT):
                                for ps, jt in zip(pss, jts):
                                    nc.tensor.matmul(
                                        ps[:],
                                        kt_sb[:, m, jt * P:(jt + 1) * P],
                                        qt[:, m, ic * NF:(ic + 1) * NF],
                                        start=(m == 0),
                                        stop=(m == MT - 1),
                                    )
                            for ps, jt in zip(pss, jts):
                                p_slice = pt[:, jt, ic * NF:(ic + 1) * NF]
                                nc.scalar.activation(p_slice, ps[:], Exp, scale=scale)
                                nc.tensor.matmul(
                                    cs_ps[:, ic * NF:(ic + 1) * NF],
                                    ones_sb[:],
                                    p_slice,
                                    start=(jt == 0),
                                    stop=(jt == JT - 1),
                                )

                recip = rpool.tile([P, HW], f32, name="recip")
                nc.vector.reciprocal(recip[:], cs_ps[:])

                # ------------- Stage C: O = V^T.T @ Pt, exchange, fuse ------
                with tc.tile_pool(name="vpool", bufs=1) as vpool, \
                     tc.tile_pool(name="kppool", bufs=1) as kppool, \
                     tc.tile_pool(name="opool", bufs=4) as opool, \
                     tc.tile_pool(name="fpool", bufs=3) as fpool:
                    vt = vpool.tile([P, JT, C], dt_c, name="vt")
                    vtd_r = vtd[:].rearrange("(jo p) c -> p jo c", p=P)
                    for jt in range(JT):
                        nc.sync.dma_start(vt[:, jt, :], vtd_r[:, jt, :])
                    keep_sb = kppool.tile([P, JT, HW], f32, name="keep_sb")

                    # send half (att rows HALF:2*HALF) first, then keep half
                    m_order = list(range(8, 16)) + list(range(8))
                    for mo, m in enumerate(m_order):
                        send = m >= 8
                        for ic in range(IC):
                            ps = psum_pool.tile([P, NF], f32, name="ps_o", tag="ps")
                            for jt in range(JT):
                                nc.tensor.matmul(
                                    ps[:],
                                    vt[:, jt, m * P:(m + 1) * P],
                                    pt[:, jt, ic * NF:(ic + 1) * NF],
                                    start=(jt == 0),
                                    stop=(jt == JT - 1),
                                )
                            t1 = opool.tile([P, NF], f32, name="t1", tag="t1")
                            nc.vector.tensor_tensor(
                                t1[:], ps[:], recip[:, ic * NF:(ic + 1) * NF],
                                mybir.AluOpType.mult,
                            )
                            if send:
                                att_sb = opool.tile([P, NF], f32, name="att_sb", tag="att_sb")
                            else:
                                att_sb = keep_sb[:, m, ic * NF:(ic + 1) * NF]
                            nc.scalar.activation(
                                att_sb[:], t1[:], Ident,
                                bias=bias_sb[:, 2 * MT + m:2 * MT + m + 1],
                            )
                            nc.sync.dma_start(
                                att_d[m * P:(m + 1) * P, ic * NF:(ic + 1) * NF],
                                att_sb[:],
                            )
                            if send:
                                g, mloc = divmod(m - 8, 4)
                                nc.sync.dma_start(
                                    cc_in[g][mloc * P:(mloc + 1) * P,
                                             ic * NF:(ic + 1) * NF],
                                    att_sb[:],
                                )
                        if mo == 3 or mo == 7:
                            g = (mo - 3) // 4
                            nc.gpsimd.collective_compute(
                                "AllGather",
                                mybir.AluOpType.bypass,
                                replica_groups=PAIRS,
                                ins=[cc_in[g][:].opt()],
                                outs=[cc_out[g][:].opt()],
                            )

                    # fusion: fusedh = a*gath[0] + b*gath[1] + c*keep
                    for m in range(8):
                        g, mloc = divmod(m, 4)
                        for ic in range(IC):
                            cols = slice(ic * NF, (ic + 1) * NF)
                            gt0 = fpool.tile([P, NF], f32, name="gt0", tag="gt0")
                            nc.sync.dma_start(
                                gt0[:], cc_out[g][0, mloc * P:(mloc + 1) * P, cols])
                            gt1 = fpool.tile([P, NF], f32, name="gt1", tag="gt1")
                            nc.sync.dma_start(
                                gt1[:], cc_out[g][1, mloc * P:(mloc + 1) * P, cols])
                            t0 = fpool.tile([P, NF], f32, name="ft0", tag="ft0")
                            nc.scalar.activation(t0[:], gt0[:], Copy,
                                                 scale=coef_sb[:, 0:1])
                            t1f = fpool.tile([P, NF], f32, name="ft1", tag="ft1")
                            nc.scalar.activation(t1f[:], gt1[:], Copy,
                                                 scale=coef_sb[:, 1:2])
                            s01 = fpool.tile([P, NF], f32, name="fs", tag="fs")
                            nc.vector.tensor_tensor(s01[:], t0[:], t1f[:],
                                                    mybir.AluOpType.add)
                            t2 = fpool.tile([P, NF], f32, name="ft2", tag="ft2")
                            nc.scalar.activation(t2[:], keep_sb[:, m, cols], Copy,
                                                 scale=coef_sb[:, 2:3])
                            fo = fpool.tile([P, NF], f32, name="fo", tag="fo")
                            nc.vector.tensor_tensor(fo[:], s01[:], t2[:],
                                                    mybir.AluOpType.add)
                            nc.sync.dma_start(
                                fusedh_d[m * P:(m + 1) * P, cols], fo[:])

    nc.compile()
    return nc


def _get_program(mode=None):
    mode = mode or DT_MODE
    if mode not in _PROGRAM_CACHE:
        _PROGRAM_CACHE[mode] = _build_program(mode)
    return _PROGRAM_CACHE[mode]


def _shard_inputs(inputs, mode=None):
    mode = mode or DT_MODE
    if mode == "bf16":
        import ml_dtypes
        np_in = ml_dtypes.bfloat16
    else:
        np_in = np.float32

    F_rgb = np.asarray(inputs["F_rgb"], dtype=np.float32).reshape(B, C, HW)
    F_ind = np.asarray(inputs["F_indices"], dtype=np.float32).reshape(B, C, HW)
    w = float(np.asarray(inputs["fusion_weight"], dtype=np.float32))

    def wt(name, perm=False):
        a = np.asarray(inputs[name], dtype=np.float32).T  # (c_in, c_out)
        if perm:  # swap output-channel halves -> [keep; send] order
            a = np.concatenate([a[:, HALF:], a[:, :HALF]], axis=1)
        return np.ascontiguousarray(a).astype(np_in)

    def bias(name, perm=False):
        a = np.asarray(inputs[name], dtype=np.float32)
        if perm:
            a = np.concatenate([a[HALF:], a[:HALF]])
        return np.ascontiguousarray(a)

    ones = np.ones((P, P), dtype=np_in)
    coef_rgb = np.tile(np.array([0.0, 1.0 - w, w], dtype=np.float32), (P, 1))
    coef_ind = np.tile(np.array([w, 0.0, 1.0 - w], dtype=np.float32), (P, 1))

    wq_rgb, wq_ind = wt("w_q_rgb"), wt("w_q_ind")
    wk_rgb, wk_ind = wt("w_k_rgb"), wt("w_k_ind")
    wv_rgb = wt("w_v_rgb", perm=True)   # used by odd cores (ind direction? no:)
    wv_ind = wt("w_v_ind")              # used by even cores (rgb att), identity perm
    bv_rgb = bias("b_v_rgb", perm=True)
    bv_ind = bias("b_v_ind")

    in_maps = []
    for b in range(B):
        xr = np.ascontiguousarray(F_rgb[b]).astype(np_in)
        xi = np.ascontiguousarray(F_ind[b]).astype(np_in)
        # core 2b (even): rgb_att -- Q from rgb, K/V from indices; keep=[0:HALF]
        in_maps.append({
            "xq": xr, "xkv": xi,
            "wq": wq_rgb, "wk": wk_ind, "wv": wv_ind,
            "bq": bias("b_q_rgb"), "bk": bias("b_k_ind"), "bv": bv_ind,
            "coef": np.ascontiguousarray(coef_rgb),
            "ones": ones,
        })
        # core 2b+1 (odd): ind_att -- Q from indices, K/V from rgb; keep=[HALF:]
        in_maps.append({
            "xq": xi, "xkv": xr,
            "wq": wq_ind, "wk": wk_rgb, "wv": wv_rgb,
            "bq": bias("b_q_ind"), "bk": bias("b_k_rgb"), "bv": bv_rgb,
            "coef": np.ascontiguousarray(coef_ind),
            "ones": ones,
        })
    return in_maps


def _assemble(inputs, results):
    fused = np.empty((B, C, H, W), dtype=np.float32)
    attention_maps = np.empty((B, 2, C, H, W), dtype=np.float32)
    for b in range(B):
        att_rgb = results[2 * b]["att"]           # canonical row order
        att_ind_p = results[2 * b + 1]["att"]     # [orig HALF:2H; orig 0:H]
        att_ind = np.concatenate([att_ind_p[HALF:], att_ind_p[:HALF]], axis=0)
        attention_maps[b, 0] = att_rgb.reshape(C, H, W)
        attention_maps[b, 1] = att_ind.reshape(C, H, W)
        fused[b, :HALF] = results[2 * b]["fusedh"].reshape(HALF, H, W)
        fused[b, HALF:] = results[2 * b + 1]["fusedh"].reshape(HALF, H, W)
    F_rgb = np.asarray(inputs["F_rgb"], dtype=np.float32)
    F_ind = np.asarray(inputs["F_indices"], dtype=np.float32)
    return fused, (F_rgb, F_ind), attention_maps


def run(inputs, mode=None, trace=False, tmpdir=None):
    nc = _get_program(mode)
    in_maps = _shard_inputs(inputs, mode)
    res = run_bass_kernel_spmd(
        nc, in_maps, core_ids=list(range(N_CORES)), trace=trace, tmpdir=tmpdir
    )
    return _assemble(inputs, res.results), res


def kernel(**inputs):
    out, _ = run(inputs)
    return out


# revision 15
# speedup vs baseline: 1.1479x; 1.0128x over previous
"""Trainium2 Bass kernel for nn_CrossAttentionModule (cross-attention fusion).

Computation (per batch b):
  rgb_att = Attn(Q=conv(F_rgb,Wq_rgb), K=conv(F_ind,Wk_ind), V=conv(F_ind,Wv_ind))
  ind_att = Attn(Q=conv(F_ind,Wq_ind), K=conv(F_rgb,Wk_rgb), V=conv(F_rgb,Wv_rgb))
  fused   = w*rgb_att + (1-w)*ind_att
  returns fused, (F_rgb, F_indices), stack([rgb_att, ind_att], 1)

Sharding: 8 cores = 4 batches x 2 attention directions. Each core computes its
(b, dir) attention output. For the fused output each pair of cores exchanges
the half of its attention output that the peer fuses (AllGather of the "send"
half), then both compute their fused half as a*gath[0] + b*gath[1] + c*keep
with host-supplied per-core coefficients (a,b,c), so the SPMD program is
uniform across cores. The V projection's output channels are host-permuted
([keep; send] order) per core parity; the host un-permutes the outputs.

Per-core compute (C=2048 channels, HW=1024 pixels, P=128):
  Q'[c,i] = sum_k WqT[k,c] Xq[k,i] + bq[c]        (out: [c-part, i-free])
  K'[c,j] likewise; V^T[j,c] = sum_k Xkv[k,j] WvT[k,c]   (V bias folded later)
  S^T[j,i] = sum_c K'[c,j] Q'[c,i]                 (psum accumulate over c)
  Pt = exp(S^T / sqrt(C))                          (unnormalized, softmax over j)
  colsum[i] broadcast to all partitions via ones[128,128] @ Pt matmuls
  O[c,i] = (sum_j V^T[j,c] Pt[j,i]) * recip_colsum[i] + bv[c]
  (exact: sum_j softmax == 1 folds the V bias into a per-partition add)

Matmuls run as float32r (FP22 truncated reads) which is full-rate on the PE
for free-dim >= 256 while keeping ~3e-4 relative accuracy.
"""

import math
import os
import sys

for _p in ("/opt/trn_rl_repo",):
    if _p not in sys.path:
        sys.path.insert(0, _p)

import numpy as np

import concourse.bass as bass
import concourse.mybir as mybir
import concourse.tile as tile
from concourse import bacc
from concourse.bass_utils import run_bass_kernel_spmd

B, C, H, W = 4, 2048, 32, 32
HW = H * W
P = 128
KT = C // P      # 16 contraction tiles
MT = C // P      # 16 output-channel tiles
JT = HW // P     # 8 pixel tiles (attention keys)
NF = 512         # matmul moving free dim
IC = HW // NF    # 2 i-chunks
N_CORES = 8
PAIRS = [[0, 1], [2, 3], [4, 5], [6, 7]]
HALF = C // 2

DT_MODE = os.environ.get("KERNEL_DT", "fp32r")  # 'fp32r' | 'bf16' | 'fp32'

_PROGRAM_CACHE = {}


def _build_program(mode: str):
    f32 = mybir.dt.float32
    if mode == "bf16":
        dt_c = mybir.dt.bfloat16
    elif mode == "fp32r":
        dt_c = mybir.dt.float32r
    elif mode == "fp32":
        dt_c = f32
    else:
        raise ValueError(mode)

    nc = bacc.Bacc(
        "TRN2", target_bir_lowering=False, debug=False, num_devices=N_CORES
    )

    xq_d = nc.dram_tensor("xq", [C, HW], dt_c, kind="ExternalInput").ap()
    xkv_d = nc.dram_tensor("xkv", [C, HW], dt_c, kind="ExternalInput").ap()
    wq_d = nc.dram_tensor("wq", [C, C], dt_c, kind="ExternalInput").ap()
    wk_d = nc.dram_tensor("wk", [C, C], dt_c, kind="ExternalInput").ap()
    wv_d = nc.dram_tensor("wv", [C, C], dt_c, kind="ExternalInput").ap()
    bq_d = nc.dram_tensor("bq", [C], f32, kind="ExternalInput").ap()
    bk_d = nc.dram_tensor("bk", [C], f32, kind="ExternalInput").ap()
    bv_d = nc.dram_tensor("bv", [C], f32, kind="ExternalInput").ap()
    coef_d = nc.dram_tensor("coef", [P, 3], f32, kind="ExternalInput").ap()
    ones_d = nc.dram_tensor("ones", [P, P], dt_c, kind="ExternalInput").ap()

    att_d = nc.dram_tensor("att", [C, HW], f32, kind="ExternalOutput").ap()
    fusedh_d = nc.dram_tensor("fusedh", [HALF, HW], f32, kind="ExternalOutput").ap()

    scale = 1.0 / math.sqrt(float(C))
    Exp = mybir.ActivationFunctionType.Exp
    Ident = mybir.ActivationFunctionType.Identity
    Copy = mybir.ActivationFunctionType.Copy

    with tile.TileContext(nc) as tc:
        with tc.tile_pool(name="dram", bufs=1, space="DRAM") as dram_pool, \
             tc.tile_pool(name="consts", bufs=1) as consts, \
             tc.tile_pool(name="psum", bufs=4, space="PSUM") as psum_pool, \
             tc.tile_pool(name="stage", bufs=4) as stage_pool:

            # DRAM spills and collective buffers
            qd = dram_pool.tile([C, HW], dt_c, name="qd")
            kd = dram_pool.tile([C, HW], dt_c, name="kd")
            vtd = dram_pool.tile([HW, C], dt_c, name="vtd")
            cc_in = [dram_pool.tile([4 * P, HW], f32, name=f"cc_in{g}")
                     for g in range(2)]
            cc_out = [dram_pool.tile([2, 4 * P, HW], f32, name=f"cc_out{g}")
                      for g in range(2)]

            # constants
            bias_sb = consts.tile([P, 3 * MT], f32, name="bias_sb")
            nc.sync.dma_start(bias_sb[:, 0:MT], bq_d.rearrange("(m p) -> p m", p=P))
            nc.sync.dma_start(bias_sb[:, MT:2 * MT], bk_d.rearrange("(m p) -> p m", p=P))
            nc.sync.dma_start(bias_sb[:, 2 * MT:3 * MT], bv_d.rearrange("(m p) -> p m", p=P))
            coef_sb = consts.tile([P, 3], f32, name="coef_sb")
            nc.sync.dma_start(coef_sb[:], coef_d[:])
            ones_sb = consts.tile([P, P], dt_c, name="ones_sb")
            nc.sync.dma_start(ones_sb[:], ones_d[:])

            # ---------------- Stage A: projections (spill to DRAM) ----------
            def projection(x_sb, w_dram, bias_col, out_dram, out_is_qk, nf_w):
                """out_is_qk: out[c_out, i]; else v-style out[j, c_out].
                nf_w: W chunk width (output channels per SBUF-resident chunk).
                W loads go through the vector queue so they don't sit behind
                the X loads on the sync queue."""
                w_r = w_dram.rearrange("(ko p) o -> p ko o", p=P)
                n_chunks = C // nf_w
                mpc = nf_w // P  # m-tiles per chunk
                with tc.tile_pool(name="wpool", bufs=2) as wpool:
                    for oq in range(n_chunks):
                        wt = wpool.tile([P, KT, nf_w], dt_c, name="wt",
                                        tag=f"wt{nf_w}")
                        nc.sync.dma_start(wt[:], w_r[:, :, oq * nf_w:(oq + 1) * nf_w])
                        if out_is_qk:
                            for mi in range(mpc):
                                m = oq * mpc + mi
                                for ic in range(IC):
                                    ps = psum_pool.tile([P, NF], f32, name="ps_a", tag="ps")
                                    for k in range(KT):
                                        nc.tensor.matmul(
                                            ps[:],
                                            wt[:, k, mi * P:(mi + 1) * P],
                                            x_sb[:, k, ic * NF:(ic + 1) * NF],
                                            start=(k == 0),
                                            stop=(k == KT - 1),
                                        )
                                    st = stage_pool.tile([P, NF], dt_c, name="st_a", tag="st")
                                    nc.scalar.activation(
                                        st[:], ps[:], Ident,
                                        bias=bias_sb[:, bias_col + m:bias_col + m + 1],
                                    )
                                    nc.gpsimd.dma_start(
                                        out_dram[m * P:(m + 1) * P, ic * NF:(ic + 1) * NF],
                                        st[:],
                                    )
                        else:
                            for jt in range(JT):
                                ps = psum_pool.tile([P, NF], f32, name="ps_v", tag="ps")
                                for k in range(KT):
                                    nc.tensor.matmul(
                                        ps[:, :nf_w],
                                        x_sb[:, k, jt * P:(jt + 1) * P],
                                        wt[:, k, :],
                                        start=(k == 0),
                                        stop=(k == KT - 1),
                                    )
                                st = stage_pool.tile([P, NF], dt_c, name="st_v", tag="st")
                                nc.scalar.activation(st[:, :nf_w], ps[:, :nf_w], Copy)
                                nc.gpsimd.dma_start(
                                    out_dram[jt * P:(jt + 1) * P, oq * nf_w:(oq + 1) * nf_w],
                                    st[:, :nf_w],
                                )

            def load_x(pool, src, name):
                x_sb = pool.tile([P, KT, HW], dt_c, name=name)
                src_r = src.rearrange("(ko p) i -> p ko i", p=P)
                for kc in range(4):  # 4 chunks of 4 k-tiles
                    nc.gpsimd.dma_start(x_sb[:, 4 * kc:4 * (kc + 1), :],
                                      src_r[:, 4 * kc:4 * (kc + 1), :])
                return x_sb

            with tc.tile_pool(name="xqpool", bufs=1) as xqpool:
                with tc.tile_pool(name="xkvpool", bufs=1) as xkvpool:
                    xkv_sb = load_x(xkvpool, xkv_d, "xkv_sb")
                    projection(xkv_sb, wk_d, MT, kd, True, NF)      # K'
                    xq_sb = load_x(xqpool, xq_d, "xq_sb")           # prefetch
                    projection(xkv_sb, wv_d, 0, vtd, False, NF // 2)  # V^T
                projection(xq_sb, wq_d, 0, qd, True, NF)        # Q'

            # ---------------- Stage B: S^T, exp, colsum ---------------------
            with tc.tile_pool(name="cspool", bufs=1, space="PSUM") as cspool, \
                 tc.tile_pool(name="ppool", bufs=1) as ppool, \
                 tc.tile_pool(name="rpool", bufs=1) as rpool:
                pt = ppool.tile([P, JT, HW], dt_c, name="pt")
                cs_ps = cspool.tile([P, HW], f32, name="cs_ps")

                with tc.tile_pool(name="qkpool", bufs=1) as qkpool:
                    qt = qkpool.tile([P, MT, HW], dt_c, name="qt")
                    kt_sb = qkpool.tile([P, MT, HW], dt_c, name="kt_sb")
                    qd_r = qd[:].rearrange("(mo p) i -> p mo i", p=P)
                    kd_r = kd[:].rearrange("(mo p) i -> p mo i", p=P)
                    # chunked reload so S matmul chains can start while streaming
                    for mc_ in range(MT // 2):
                        sl = slice(2 * mc_, 2 * (mc_ + 1))
                        nc.sync.dma_start(qt[:, sl, :], qd_r[:, sl, :])
                        nc.sync.dma_start(kt_sb[:, sl, :], kd_r[:, sl, :])

                    for ic in range(IC):
                        for pr in range(JT // 2):  # jt pairs
                            jts = (2 * pr, 2 * pr + 1)
                            pss = [
                                psum_pool.tile([P, NF], f32, name="ps_s", tag="ps")
                                for _ in jts
                            ]
                            for m in range(MT):
                                for ps, jt in zip(pss, jts):
                                    nc.tensor.matmul(
                                        ps[:],
                                        kt_sb[:, m, jt * P:(jt + 1) * P],
                                        qt[:, m, ic * NF:(ic + 1) * NF],
                                        start=(m == 0),
                                        stop=(m == MT - 1),
                                    )
                            for ps, jt in zip(pss, jts):
                                p_slice = pt[:, jt, ic * NF:(ic + 1) * NF]
                                nc.scalar.activation(p_slice, ps[:], Exp, scale=scale)
                                nc.tensor.matmul(
                                    cs_ps[:, ic * NF:(ic + 1) * NF],
                                    ones_sb[:],
                                    p_slice,
                                    start=(jt == 0),
                                    stop=(jt == JT - 1),
                                )

                recip = rpool.tile([P, HW], f32, name="recip")
                nc.vector.reciprocal(recip[:], cs_ps[:])

                # ------------- Stage C: O = V^T.T @ Pt, exchange, fuse ------
                with tc.tile_pool(name="vpool", bufs=1) as vpool, \
                     tc.tile_pool(name="kppool", bufs=1) as kppool, \
                     tc.tile_pool(name="opool", bufs=4) as opool, \
                     tc.tile_pool(name="fpool", bufs=3) as fpool:
                    vt = vpool.tile([P, JT, C], dt_c, name="vt")
                    vtd_r = vtd[:].rearrange("(jo p) c -> p jo c", p=P)
                    for jc in range(JT // 2):
                        sl = slice(2 * jc, 2 * (jc + 1))
                        nc.sync.dma_start(vt[:, sl, :], vtd_r[:, sl, :])
                    keep_sb = kppool.tile([P, JT, HW], f32, name="keep_sb")

                    # send half (att rows HALF:2*HALF) first, then keep half
                    m_order = list(range(8, 16)) + list(range(8))
                    for mo, m in enumerate(m_order):
                        send = m >= 8
                        for ic in range(IC):
                            ps = psum_pool.tile([P, NF], f32, name="ps_o", tag="ps")
                            for jt in range(JT):
                                nc.tensor.matmul(
                                    ps[:],
                                    vt[:, jt, m * P:(m + 1) * P],
                                    pt[:, jt, ic * NF:(ic + 1) * NF],
                                    start=(jt == 0),
                                    stop=(jt == JT - 1),
                                )
                            t1 = opool.tile([P, NF], f32, name="t1", tag="t1")
                            nc.vector.tensor_tensor(
                                t1[:], ps[:], recip[:, ic * NF:(ic + 1) * NF],
                                mybir.AluOpType.mult,
                            )
                            if send:
                                att_sb = opool.tile([P, NF], f32, name="att_sb", tag="att_sb")
                            else:
                                att_sb = keep_sb[:, m, ic * NF:(ic + 1) * NF]
                            nc.scalar.activation(
                                att_sb[:], t1[:], Ident,
                                bias=bias_sb[:, 2 * MT + m:2 * MT + m + 1],
                            )
                            nc.gpsimd.dma_start(
                                att_d[m * P:(m + 1) * P, ic * NF:(ic + 1) * NF],
                                att_sb[:],
                            )
                            if send:
                                g, mloc = divmod(m - 8, 4)
                                nc.gpsimd.dma_start(
                                    cc_in[g][mloc * P:(mloc + 1) * P,
                                             ic * NF:(ic + 1) * NF],
                                    att_sb[:],
                                )
                        if mo == 3 or mo == 7:
                            g = (mo - 3) // 4
                            nc.gpsimd.collective_compute(
                                "AllGather",
                                mybir.AluOpType.bypass,
                                replica_groups=PAIRS,
                                ins=[cc_in[g][:].opt()],
                                outs=[cc_out[g][:].opt()],
                            )

                    # fusion: fusedh = a*gath[0] + b*gath[1] + c*keep
                    for m in range(8):
                        g, mloc = divmod(m, 4)
                        for ic in range(IC):
                            cols = slice(ic * NF, (ic + 1) * NF)
                            gt0 = fpool.tile([P, NF], f32, name="gt0", tag="gt0")
                            nc.sync.dma_start(
                                gt0[:], cc_out[g][0, mloc * P:(mloc + 1) * P, cols])
                            gt1 = fpool.tile([P, NF], f32, name="gt1", tag="gt1")
                            nc.sync.dma_start(
                                gt1[:], cc_out[g][1, mloc * P:(mloc + 1) * P, cols])
                            t0 = fpool.tile([P, NF], f32, name="ft0", tag="ft0")
                            nc.scalar.activation(t0[:], gt0[:], Copy,
                                                 scale=coef_sb[:, 0:1])
                            t1f = fpool.tile([P, NF], f32, name="ft1", tag="ft1")
                            nc.scalar.activation(t1f[:], gt1[:], Copy,
                                                 scale=coef_sb[:, 1:2])
                            s01 = fpool.tile([P, NF], f32, name="fs", tag="fs")
                            nc.vector.tensor_tensor(s01[:], t0[:], t1f[:],
                                                    mybir.AluOpType.add)
                            t2 = fpool.tile([P, NF], f32, name="ft2", tag="ft2")
                            nc.scalar.activation(t2[:], keep_sb[:, m, cols], Copy,
                                                 scale=coef_sb[:, 2:3])
                            fo = fpool.tile([P, NF], f32, name="fo", tag="fo")
                            nc.vector.tensor_tensor(fo[:], s01[:], t2[:],
                                                    mybir.AluOpType.add)
                            nc.gpsimd.dma_start(
                                fusedh_d[m * P:(m + 1) * P, cols], fo[:])

    nc.compile()
    return nc


def _get_program(mode=None):
    mode = mode or DT_MODE
    if mode not in _PROGRAM_CACHE:
        _PROGRAM_CACHE[mode] = _build_program(mode)
    return _PROGRAM_CACHE[mode]


def _shard_inputs(inputs, mode=None):
    mode = mode or DT_MODE
    if mode == "bf16":
        import ml_dtypes
        np_in = ml_dtypes.bfloat16
    else:
        np_in = np.float32

    F_rgb = np.asarray(inputs["F_rgb"], dtype=np.float32).reshape(B, C, HW)
    F_ind = np.asarray(inputs["F_indices"], dtype=np.float32).reshape(B, C, HW)
    w = float(np.asarray(inputs["fusion_weight"], dtype=np.float32))

    def wt(name, perm=False):
        a = np.asarray(inputs[name], dtype=np.float32).T  # (c_in, c_out)
        if perm:  # swap output-channel halves -> [keep; send] order
            a = np.concatenate([a[:, HALF:], a[:, :HALF]], axis=1)
        return np.ascontiguousarray(a).astype(np_in)

    def bias(name, perm=False):
        a = np.asarray(inputs[name], dtype=np.float32)
        if perm:
            a = np.concatenate([a[HALF:], a[:HALF]])
        return np.ascontiguousarray(a)

    ones = np.ones((P, P), dtype=np_in)
    coef_rgb = np.tile(np.array([0.0, 1.0 - w, w], dtype=np.float32), (P, 1))
    coef_ind = np.tile(np.array([w, 0.0, 1.0 - w], dtype=np.float32), (P, 1))

    wq_rgb, wq_ind = wt("w_q_rgb"), wt("w_q_ind")
    wk_rgb, wk_ind = wt("w_k_rgb"), wt("w_k_ind")
    wv_rgb = wt("w_v_rgb", perm=True)   # used by odd cores (ind direction? no:)
    wv_ind = wt("w_v_ind")              # used by even cores (rgb att), identity perm
    bv_rgb = bias("b_v_rgb", perm=True)
    bv_ind = bias("b_v_ind")

    in_maps = []
    for b in range(B):
        xr = np.ascontiguousarray(F_rgb[b]).astype(np_in)
        xi = np.ascontiguousarray(F_ind[b]).astype(np_in)
        # core 2b (even): rgb_att -- Q from rgb, K/V from indices; keep=[0:HALF]
        in_maps.append({
            "xq": xr, "xkv": xi,
            "wq": wq_rgb, "wk": wk_ind, "wv": wv_ind,
            "bq": bias("b_q_rgb"), "bk": bias("b_k_ind"), "bv": bv_ind,
            "coef": np.ascontiguousarray(coef_rgb),
            "ones": ones,
        })
        # core 2b+1 (odd): ind_att -- Q from indices, K/V from rgb; keep=[HALF:]
        in_maps.append({
            "xq": xi, "xkv": xr,
            "wq": wq_ind, "wk": wk_rgb, "wv": wv_rgb,
            "bq": bias("b_q_ind"), "bk": bias("b_k_rgb"), "bv": bv_rgb,
            "coef": np.ascontiguousarray(coef_ind),
            "ones": ones,
        })
    return in_maps


def _assemble(inputs, results):
    fused = np.empty((B, C, H, W), dtype=np.float32)
    attention_maps = np.empty((B, 2, C, H, W), dtype=np.float32)
    for b in range(B):
        att_rgb = results[2 * b]["att"]           # canonical row order
        att_ind_p = results[2 * b + 1]["att"]     # [orig HALF:2H; orig 0:H]
        att_ind = np.concatenate([att_ind_p[HALF:], att_ind_p[:HALF]], axis=0)
        attention_maps[b, 0] = att_rgb.reshape(C, H, W)
        attention_maps[b, 1] = att_ind.reshape(C, H, W)
        fused[b, :HALF] = results[2 * b]["fusedh"].reshape(HALF, H, W)
        fused[b, HALF:] = results[2 * b + 1]["fusedh"].reshape(HALF, H, W)
    F_rgb = np.asarray(inputs["F_rgb"], dtype=np.float32)
    F_ind = np.asarray(inputs["F_indices"], dtype=np.float32)
    return fused, (F_rgb, F_ind), attention_maps


def run(inputs, mode=None, trace=False, tmpdir=None):
    nc = _get_program(mode)
    in_maps = _shard_inputs(inputs, mode)
    res = run_bass_kernel_spmd(
        nc, in_maps, core_ids=list(range(N_CORES)), trace=trace, tmpdir=tmpdir
    )
    return _assemble(inputs, res.results), res


def kernel(**inputs):
    out, _ = run(inputs)
    return out
